# revision 1
# baseline (speedup 1.0000x reference)
"""Kohonen SOM distance-matrix kernel for Trainium2 (Bass/Tile).

Computes sqrt(max(||x||^2 + ||w||^2 - 2 x.w, 0)) for x [32768, 256] against a
codebook w [2500, 256] -> out [32768, 2500], data-parallel over 8 NeuronCores
(batch sharded, codebook replicated).

Per core (batch shard of 4096 rows):
  - Host preps transposed operands:
      xt [256, 4096] = x.T (float32r), wt [256, 2500] = (-2 w).T (float32r)
      xsq [128, 32] = ||x||^2 tiled [row-in-tile, m-tile], wsq [1, 2500]
  - TensorE (float32r: FP22 multiply, FP32 accumulate, full rate for N>=256)
    computes cross = -2 x.w into bank-aligned PSUM groups; x is the stationary
    operand so output partitions = batch rows (contiguous 10KB-row stores).
  - VectorE adds broadcast ||w||^2 (PSUM -> SBUF), ScalarE computes
    sqrt(t + ||x||^2) in-place in SBUF via per-partition bias, DMA out.
  - PE warm-up matmuls run during the input-load phase to engage the HAM
    clock un-throttle (1.2 -> 2.4 GHz) before real compute starts.
"""

import numpy as np

N_CORES = 8
BATCH = 32768
BS = BATCH // N_CORES  # 4096 rows per core
N = 2500
D = 256
M_TILE = 128
M_TILES = BS // M_TILE  # 32

DEFAULT_CFG = {
    "x_chunk": 512,  # columns per x-load chunk
    "groups": (512, 512, 512, 512, 452),  # PSUM group widths (1 bank each)
    "fuse_act": False,  # sqrt per group (pipelines tighter than per-m)
    "store_alt": False,  # all stores on the SP queue (ACT queue carries loads)
    "last_split": False,
    "warm_mm": 16,  # PE warm-up matmuls (~6.8us cold)
    "psum_bufs": 7,
}

_CACHE = {}


def _build_bass(cfg=None):
    import concourse.mybir as mybir
    from concourse import bacc
    from concourse.tile import TileContext

    cfg = {**DEFAULT_CFG, **(cfg or {})}
    x_chunk = cfg["x_chunk"]
    x_chunks = BS // x_chunk
    mt_per_chunk = x_chunk // M_TILE
    groups = []
    g0 = 0
    for gw in cfg["groups"]:
        groups.append((g0, gw))
        g0 += gw
    assert g0 == N, groups

    f32 = mybir.dt.float32
    f32r = mybir.dt.float32r

    nc = bacc.Bacc("TRN2", target_bir_lowering=False, debug=False)
    xt = nc.dram_tensor("xt", [D, BS], f32r, kind="ExternalInput")
    wt = nc.dram_tensor("wt", [D, N], f32r, kind="ExternalInput")
    xsq_d = nc.dram_tensor("xsq", [M_TILE, M_TILES], f32, kind="ExternalInput")
    wsq_d = nc.dram_tensor("wsq", [1, N], f32, kind="ExternalInput")
    out = nc.dram_tensor("out", [BS, N], f32, kind="ExternalOutput")

    with TileContext(nc) as tc:
        with (
            tc.tile_pool(name="wpool", bufs=1) as wpool,
            tc.tile_pool(name="xpool", bufs=1) as xpool,
            tc.tile_pool(name="bpool", bufs=1) as bpool,
            tc.tile_pool(name="opool", bufs=4) as opool,
            tc.tile_pool(name="pp", bufs=cfg["psum_bufs"], space="PSUM") as pp,
            tc.tile_pool(name="pwarm", bufs=1, space="PSUM") as pwarm,
        ):
            # --- PE warm-up: no DMA deps, issues at t=0 while inputs load.
            warm_src = bpool.tile([M_TILE, 512], mybir.dt.bfloat16)
            nc.vector.memset(warm_src, 0.0)
            warm_ps = pwarm.tile([M_TILE, 512], f32)
            for _ in range(cfg["warm_mm"]):
                nc.tensor.matmul(
                    warm_ps, lhsT=warm_src[:, :M_TILE], rhs=warm_src, start=True,
                    stop=True,
                )

            # --- input loads. Small/early loads go on the SP HWDGE queue
            # (idle at start; its first store only becomes ready ~10us in),
            # x chunks on the ACT queue. wsq first: the gpsimd broadcast
            # (~6us) overlaps the w loads.
            wsq_row = bpool.tile([1, N], f32)
            nc.sync.dma_start(wsq_row, wsq_d[:, :])
            wsq_bc = bpool.tile([M_TILE, N], f32)
            nc.gpsimd.partition_broadcast(wsq_bc, wsq_row[0:1, :])
            xsq = bpool.tile([M_TILE, M_TILES], f32)
            nc.sync.dma_start(xsq, xsq_d[:, :])
            w_sb = []
            for ki in range(2):
                wk = wpool.tile([128, N], f32r, name=f"wk{ki}")
                nc.sync.dma_start(wk, wt[ki * 128 : (ki + 1) * 128, :])
                w_sb.append(wk)

            x_sb = [[None] * x_chunks for _ in range(2)]
            for ci in range(x_chunks):
                cs = slice(ci * x_chunk, (ci + 1) * x_chunk)
                for ki in range(2):
                    xc = xpool.tile([128, x_chunk], f32r, name=f"x{ki}_{ci}")
                    nc.scalar.dma_start(xc, xt[ki * 128 : (ki + 1) * 128, cs])
                    x_sb[ki][ci] = xc

            # --- main loop over batch tiles. Matmuls write bank-aligned
            # <=512-col slices; DVE adds amortize overhead over whole groups.
            for m in range(M_TILES):
                split = cfg["last_split"] and m == M_TILES - 1
                ms = slice(m * M_TILE, (m + 1) * M_TILE)
                mo = slice(
                    (m % mt_per_chunk) * M_TILE, (m % mt_per_chunk + 1) * M_TILE
                )
                ot = opool.tile([M_TILE, N], f32, name="ot")
                for g0, gw in groups:
                    gs = slice(g0, g0 + gw)
                    ps = pp.tile([M_TILE, gw], f32, name="ps")
                    for j in range(0, gw, 512):
                        jw = min(512, gw - j)
                        ns = slice(g0 + j, g0 + j + jw)
                        for ki in range(2):
                            nc.tensor.matmul(
                                ps[:, j : j + jw],
                                lhsT=x_sb[ki][m // mt_per_chunk][:, mo],
                                rhs=w_sb[ki][:, ns],
                                start=(ki == 0),
                                stop=(ki == 1),
                            )
                    # t = cross + ||w||^2  (PSUM -> SBUF; frees the PSUM banks
                    # with a single-consumer dep for PE)
                    nc.vector.tensor_add(ot[:, gs], ps, wsq_bc[:, gs])
                    if split or not cfg["fuse_act"]:
                        nc.scalar.activation(
                            ot[:, gs], ot[:, gs],
                            mybir.ActivationFunctionType.Sqrt,
                            bias=xsq[:, m : m + 1], scale=1.0,
                        )
                    if split:
                        nc.sync.dma_start(out[ms, gs], ot[:, gs])
                if not split:
                    if cfg["fuse_act"]:
                        # out = sqrt(t + ||x||^2) in-place in SBUF, one op per m
                        nc.scalar.activation(
                            ot,
                            ot,
                            mybir.ActivationFunctionType.Sqrt,
                            bias=xsq[:, m : m + 1],
                            scale=1.0,
                        )
                    if cfg.get("store_split"):
                        # column-split every store across both HWDGE queues:
                        # the SP half always flows; the ACT half shares with
                        # input loads only during the first ~12us
                        sp = cfg.get("split_at", 1280)
                        nc.sync.dma_start(out[ms, :sp], ot[:, :sp])
                        nc.scalar.dma_start(out[ms, sp:], ot[:, sp:])
                    else:
                        # alternate stores across the two queues (SP + ACT)
                        eng = (
                            nc.sync
                            if (m % 2 == 0 or not cfg["store_alt"])
                            else nc.scalar
                        )
                        eng.dma_start(out[ms, :], ot)

    nc.finalize()
    return nc


def _prep_inputs(x, weights):
    x = np.ascontiguousarray(np.asarray(x, dtype=np.float32))
    w = np.ascontiguousarray(np.asarray(weights, dtype=np.float32))
    assert x.shape == (BATCH, D), x.shape
    assert w.shape == (N, D), w.shape

    xt = np.ascontiguousarray(x.T)
    wt = np.ascontiguousarray((-2.0 * w).T)
    xsq = np.einsum("bd,bd->b", x, x)
    wsq = np.einsum("nd,nd->n", w, w)

    in_maps = []
    for c in range(N_CORES):
        bs = slice(c * BS, (c + 1) * BS)
        in_maps.append(
            {
                "xt": np.ascontiguousarray(xt[:, bs]),
                "wt": wt,
                "xsq": np.ascontiguousarray(xsq[bs].reshape(M_TILES, M_TILE).T),
                "wsq": np.ascontiguousarray(wsq[None, :]),
            }
        )
    return in_maps


def run(x, weights, trace=False, nc=None, **kwargs):
    from concourse.bass_utils import run_bass_kernel_spmd

    if nc is None:
        if "nc" not in _CACHE:
            _CACHE["nc"] = _build_bass()
        nc = _CACHE["nc"]
    in_maps = _prep_inputs(x, weights)
    res = run_bass_kernel_spmd(
        nc, in_maps, core_ids=list(range(N_CORES)), trace=trace, **kwargs
    )
    out = np.concatenate([res.results[c]["out"] for c in range(N_CORES)], axis=0)
    return out, res


def _get_runner():
    """Build + jit the SPMD executable once; reuse across kernel() calls."""
    if "runner" in _CACHE:
        return _CACHE["runner"]

    import jax
    import concourse.mybir as mybir
    from concourse import bass2jax
    from jax.sharding import Mesh, PartitionSpec
    from jax.experimental.shard_map import shard_map

    bass2jax.install_neuronx_cc_hook()
    if "nc" not in _CACHE:
        _CACHE["nc"] = _build_bass()
    nc = _CACHE["nc"]

    partition_name = (
        nc.partition_id_tensor.name if nc.partition_id_tensor else None
    )
    in_names, out_names, out_avals, zero_templates = [], [], [], []
    for alloc in nc.m.functions[0].allocations:
        if not isinstance(alloc, mybir.MemoryLocationSet):
            continue
        name = alloc.memorylocations[0].name
        if alloc.kind == "ExternalInput":
            if name != partition_name:
                in_names.append(name)
        elif alloc.kind == "ExternalOutput":
            out_names.append(name)
            shape = tuple(alloc.tensor_shape)
            dtype = mybir.dt.np(alloc.dtype)
            out_avals.append(jax.core.ShapedArray(shape, dtype))
            zero_templates.append((shape, dtype))
    n_params = len(in_names)
    n_outs = len(out_names)
    all_names = in_names + out_names
    if partition_name is not None:
        all_names = all_names + [partition_name]
    donate = tuple(range(n_params, n_params + n_outs))

    def _body(*args):
        operands = list(args)
        if partition_name is not None:
            operands.append(bass2jax.partition_id_tensor())
        outs = bass2jax._bass_exec_p.bind(
            *operands,
            out_avals=tuple(out_avals),
            in_names=tuple(all_names),
            out_names=tuple(out_names),
            lowering_input_output_aliases=(),
            sim_require_finite=True,
            sim_require_nnan=True,
            nc=nc,
        )
        return tuple(outs)

    devices = jax.devices()[:N_CORES]
    mesh = Mesh(np.asarray(devices), ("core",))
    specs = (PartitionSpec("core"),) * (n_params + n_outs)
    sharded = jax.jit(
        shard_map(
            _body, mesh=mesh, in_specs=specs, out_specs=specs[:n_outs],
            check_rep=False,
        ),
        donate_argnums=donate,
        keep_unused=True,
    )

    def runner(in_maps):
        concat_in = [
            np.concatenate([m[name] for m in in_maps], axis=0)
            for name in in_names
        ]
        concat_zeros = [
            np.zeros((N_CORES * s[0], *s[1:]), d) for s, d in zero_templates
        ]
        out_arrs = sharded(*concat_in, *concat_zeros)
        return np.asarray(out_arrs[out_names.index("out")])

    _CACHE["runner"] = runner
    return runner


def kernel(x, weights):
    runner = _get_runner()
    in_maps = _prep_inputs(x, weights)
    return runner(in_maps)

